# revision 2
# baseline (speedup 1.0000x reference)
"""Trainium2 Bass kernel for GQA attention (B=2, S=2048, D=2048, H=16, KVH=4).

Sharding: 8 cores = (batch b in {0,1}) x (kv-group g in {0..3}).
Core c = b*4 + g computes q-heads 4g..4g+3 against kv-head g for batch b,
producing a partial output projection res_partial.T = [e=2048, s=2048];
host sums the 4 partials per batch.

v2 layout (vs v1):
  - all matmul operands in bf16 (same 1 cycle/row PE rate as fp32r but with
    no >=256-wide requirement, half the HBM traffic and half the SBUF);
    all accumulations stay fp32 in PSUM.  rel-err budget is 2e-2; measured
    bf16 path error is ~1e-3.
  - Wq and Wo persist in SBUF (no per-chunk re-DMA: 24 MB less HBM traffic).
  - attention + output-projection for q-chunk qc are interleaved right after
    projection chunk sc==qc, so the PE never idles waiting on x/w DMAs and
    the p-state ramp stays at full clock.
  - k is computed in the FIRST projection pass so kfin is ready well before
    the attention blocks that need it.
  - causal diagonal blocks restrict the q-range to the valid triangle
    (saves ~15% of attention matmul/exp work); diag and off-diag blocks are
    interleaved so exp/mask latency on short diag blocks hides under the
    full-width ones.
  - psum->sbuf copies (q_raw, v, o-proj result) run on ACT (Identity, same
    table as Exp/Abs_rsqrt -> no table reloads beyond 2 per chunk, which are
    prefetched via a dummy scratch activation).
  - PV/denominator matmuls run two blocks behind the score matmuls so
    exp (ACT) + causal mask (Pool) latency is fully hidden.
"""

import sys

sys.path.insert(0, "/opt/trn_rl_repo")

from contextlib import ExitStack

import numpy as np

import concourse.bass as bass
import concourse.tile as tile
from concourse import bass_isa
from concourse import bacc, mybir
from concourse import bass_utils

B, S, D = 2, 2048, 2048
H, KVH = 16, 4
HD = 128               # head dim
GQ = 4                 # q heads per core
SL = GQ * HD           # 512: q-head slice width per core
NCORES = 8
SC = S // 512          # 4 s-chunks of 512
KC = D // 128          # 16 d-chunks of 128
ROPE_BASE = 10000.0
EPS = 1.1920929e-07
F32 = mybir.dt.float32
F32R = mybir.dt.float32r
BF16 = mybir.dt.bfloat16
AF = mybir.ActivationFunctionType

_COMPILED_NC = None
_LAST_IN_MAPS = None


def _build_body(tc):
    nc = tc.nc
    ctx = ExitStack()
    ctx.enter_context(nc.allow_low_precision(reason="bf16 matmul operand tiles"))

    xT = nc.dram_tensor("xT", [D, S], BF16, kind="ExternalInput").ap()
    wqT = nc.dram_tensor("wqT", [D, SL], BF16, kind="ExternalInput").ap()
    wkT = nc.dram_tensor("wkT", [D, HD], BF16, kind="ExternalInput").ap()
    wvT = nc.dram_tensor("wvT", [D, HD], BF16, kind="ExternalInput").ap()
    woA = nc.dram_tensor("woA", [SL, D], BF16, kind="ExternalInput").ap()
    csd = nc.dram_tensor("csd", [128, S], BF16, kind="ExternalInput").ap()
    snd = nc.dram_tensor("snd", [128, S], BF16, kind="ExternalInput").ap()
    onesd = nc.dram_tensor("onesd", [128, 1], BF16, kind="ExternalInput").ap()
    maskd = nc.dram_tensor("maskd", [128, 512], BF16, kind="ExternalInput").ap()
    identd = nc.dram_tensor("identd", [128, 128], BF16, kind="ExternalInput").ap()
    bqkd = nc.dram_tensor("bqkd", [128, GQ + 1], F32, kind="ExternalInput").ap()
    sced = nc.dram_tensor("sced", [128, 2 * (GQ + 1)], F32,
                          kind="ExternalInput").ap()
    resT = nc.dram_tensor("resT", [D, S], F32, kind="ExternalOutput").ap()

    persist = ctx.enter_context(tc.tile_pool(name="persist", bufs=1))
    xpool = ctx.enter_context(tc.tile_pool(name="xpool", bufs=8))
    bpool = ctx.enter_context(tc.tile_pool(name="bpool", bufs=2))
    rowp = ctx.enter_context(tc.tile_pool(name="rowp", bufs=2))
    expp = ctx.enter_context(tc.tile_pool(name="expp", bufs=6))
    otp = ctx.enter_context(tc.tile_pool(name="otp", bufs=1))
    resp = ctx.enter_context(tc.tile_pool(name="resp", bufs=2))
    vtp = ctx.enter_context(tc.tile_pool(name="vtp", bufs=2))
    psA = ctx.enter_context(tc.tile_pool(name="psA", bufs=3, space="PSUM"))
    psS = ctx.enter_context(tc.tile_pool(name="psS", bufs=2, space="PSUM"))
    psO = ctx.enter_context(tc.tile_pool(name="psO", bufs=2, space="PSUM"))
    psD = ctx.enter_context(tc.tile_pool(name="psD", bufs=1, space="PSUM"))

    # ---- persistent tiles ----
    cs_sb = persist.tile([128, S], BF16, name="cs_sb")
    sn_sb = persist.tile([128, S], BF16, name="sn_sb")
    wk_sb = persist.tile([128, KC, HD], BF16, name="wk_sb")
    wv_sb = persist.tile([128, KC, HD], BF16, name="wv_sb")
    wq_sb = persist.tile([128, KC, SL], BF16, name="wq_sb")
    wo_sb = persist.tile([128, 8, GQ, 256], BF16, name="wo_sb")
    ident = persist.tile([128, 128], BF16, name="ident")
    ones_col = persist.tile([128, 1], BF16, name="ones_col")
    mask_sb = persist.tile([128, 512], BF16, name="mask_sb")
    bqcols = persist.tile([128, GQ + 1], F32, name="bqcols")
    sce = persist.tile([128, 2 * (GQ + 1)], F32, name="sce")
    scratch = persist.tile([1, 1], F32, name="scratch")
    qfin = [
        persist.tile([128, S], BF16, name=f"qfin{h}", tag=f"qfin{h}") for h in range(GQ)
    ]
    kfin = persist.tile([128, S], BF16, name="kfin")
    v_sb = [
        persist.tile([128, HD], BF16, name=f"vsb{i}", tag=f"vsb{i}") for i in range(KC)
    ]

    # ---- DMA helpers (SP ring = critical input path) ----
    def emit_wk_dma(i):
        nc.sync.dma_start(
            wk_sb[:, 4 * i : 4 * (i + 1), :],
            bass.AP(tensor=wkT.tensor, offset=4 * i * 128 * HD,
                    ap=[[HD, 128], [128 * HD, 4], [1, HD]]))

    def emit_wq_dma(i, half):
        # kc group i, head columns [256*half, 256*half+256)
        nc.sync.dma_start(
            wq_sb[:, 4 * i : 4 * (i + 1), 256 * half : 256 * half + 256],
            bass.AP(tensor=wqT.tensor, offset=4 * i * 128 * SL + 256 * half,
                    ap=[[SL, 128], [128 * SL, 4], [1, 256]]))

    def emit_xq_dma(xt, sc, kp):
        nc.sync.dma_start(
            xt,
            bass.AP(tensor=xT.tensor, offset=kp * 256 * S + sc * 512,
                    ap=[[S, 128], [128 * S, 2], [1, 512]]))

    def emit_x_chunk(sc):
        xq = []
        if sc == 0:
            emit_wk_dma(0)
            for kp in range(8):
                xt = xpool.tile([128, 2, 512], BF16, name=f"xq0_{kp}", tag="xq",
                                bufs=8)
                if kp == 0:
                    for c2 in range(2):
                        nc.sync.dma_start(
                            xt[:, c2, :],
                            bass.AP(tensor=xT.tensor, offset=c2 * 128 * S,
                                    ap=[[S, 128], [1, 512]]))
                        if c2 == 0:
                            emit_wq_dma(0, 0)
                else:
                    emit_xq_dma(xt, 0, kp)
                xq.append(xt)
                if 1 <= kp <= 3:
                    emit_wk_dma(kp)
                    emit_wq_dma(kp, 0)
            nc.sync.dma_start(bqcols, bqkd)
            nc.sync.dma_start(sce, sced)
            nc.sync.dma_start(cs_sb[:, 0:1024], csd[:, 0:1024])
            nc.sync.dma_start(sn_sb[:, 0:1024], snd[:, 0:1024])
            emit_wq_dma(0, 1)
            emit_wq_dma(1, 1)
            nc.sync.dma_start(cs_sb[:, 1024:2048], csd[:, 1024:2048])
            nc.sync.dma_start(sn_sb[:, 1024:2048], snd[:, 1024:2048])
            nc.sync.dma_start(ident, identd)
            emit_wq_dma(2, 1)
            emit_wq_dma(3, 1)
            nc.sync.dma_start(wv_sb, wvT.rearrange("(kc p) h -> p kc h", p=128))
            nc.sync.dma_start(ones_col, onesd)
            nc.sync.dma_start(mask_sb, maskd)
        else:
            for kp in range(8):
                xt = xpool.tile([128, 2, 512], BF16, name=f"xq{sc}_{kp}", tag="xq",
                                bufs=8)
                emit_xq_dma(xt, sc, kp)
                xq.append(xt)
        return xq

    def emit_wo_dmas():
        for etg in range(8):
            nc.sync.dma_start(
                wo_sb[:, etg],
                bass.AP(tensor=woA.tensor, offset=etg * 256,
                        ap=[[D, 128], [128 * D, GQ], [1, 256]]))

    # ---- stage B: bias, rms-norm, rope.  Split in two so both psums of a
    # pass are drained (head) before the long rope chains run (tail), which
    # frees the psA slots for the next pass ~1.5us earlier. ----
    def stage_b_head(et, psum_p):
        is_q = et < GQ
        bias_col = bqcols[:, et : et + 1] if is_q else bqcols[:, GQ : GQ + 1]
        q_raw = bpool.tile([128, 512], BF16, tag="qraw", bufs=2)
        nc.scalar.activation(q_raw, psum_p, AF.Identity, bias=bias_col)
        # Square is in the same ACT table set as Identity/Exp, and reading
        # q_raw (not the psum) frees the psA slot after a single ACT op
        sq = bpool.tile([128, 512], F32, tag="sq", bufs=2)
        nc.scalar.activation(sq, q_raw, AF.Square)
        return q_raw, sq

    def stage_b_tail(et, sc, q_raw, sq):
        is_q = et < GQ
        ssr = bpool.tile([128, 512], F32, tag="ssr", bufs=1)
        nc.gpsimd.partition_all_reduce(ssr, sq, 128, bass_isa.ReduceOp.add)
        # g*rsqrt(ss/HD + eps) == rsqrt(ss*A + B) with A=1/(HD g^2), B=eps/g^2
        scale_sb = bpool.tile([128, 512], BF16, tag="scl", bufs=1)
        nc.scalar.activation(scale_sb, ssr, AF.Abs_reciprocal_sqrt,
                             bias=sce[:, GQ + 1 + et : GQ + 2 + et],
                             scale=sce[:, et : et + 1])
        # rope: swap halves via sbuf->sbuf DMA (sn rows 64..127 hold -sin)
        sw = bpool.tile([128, 512], BF16, tag="sw", bufs=2)
        nc.sync.dma_start(sw[0:64, :], q_raw[64:128, :])
        nc.sync.dma_start(sw[64:128, :], q_raw[0:64, :])
        t1 = bpool.tile([128, 512], BF16, tag="t1", bufs=2)
        nc.vector.tensor_mul(t1, q_raw, cs_sb[:, sc * 512 : (sc + 1) * 512])
        t2 = bpool.tile([128, 512], BF16, tag="t2", bufs=1)
        nc.vector.tensor_mul(t2, sw, sn_sb[:, sc * 512 : (sc + 1) * 512])
        nc.vector.tensor_add(t1, t1, t2)
        dst = qfin[et] if is_q else kfin
        nc.vector.tensor_mul(dst[:, sc * 512 : (sc + 1) * 512], t1, scale_sb)

    # ---- stage A: projection chunk (passes: (k,q0), (q1,q2), (q3,v)) ----
    pending_vts = []

    def proj_chunk(sc, xq):
        for pi, grp in enumerate(((GQ, 0), (1, 2), (3, GQ + 1))):
            psums = {}
            for et in grp:
                psums[et] = psA.tile([128, 512], F32, tag="pA",
                                     name=f"psA{sc}_{pi}_{et}")
            for kq in range(4):
                for kc4 in range(4):
                    kc = kq * 4 + kc4
                    start = kc == 0
                    stop = kc == KC - 1
                    xsl = xq[kc // 2][:, kc % 2, :]
                    for et in grp:
                        if et < GQ:
                            lhsT = wq_sb[:, kc, et * 128 : (et + 1) * 128]
                        elif et == GQ:
                            lhsT = wk_sb[:, kc, :]
                        else:
                            lhsT = wv_sb[:, kc, :]
                        nc.tensor.matmul(psums[et], lhsT, xsl,
                                         start=start, stop=stop)
            # drain the v psum first (ACT) so the PE transposes that feed
            # attention never wait behind stage_b's DVE chain; then both
            # heads (psum readers) before both tails (rope chains)
            heads = []
            for et in sorted(grp, key=lambda e: e != GQ + 1):
                if et <= GQ:
                    heads.append((et, stage_b_head(et, psums[et])))
                else:
                    vt = vtp.tile([128, 512], BF16, tag="vt", bufs=2)
                    nc.scalar.activation(vt, psums[et], AF.Identity)
                    pending_vts.append((vt, sc))
            for et, (q_raw, sq) in heads:
                stage_b_tail(et, sc, q_raw, sq)

    def flush_vts():
        while pending_vts:
            vt, vsc = pending_vts.pop(0)
            for j in range(4):
                stile = vsc * 4 + j
                pst = psS.tile([128, 128], BF16, tag="pS", name=f"pst{vsc}_{j}")
                nc.tensor.transpose(pst, vt[:, j * 128 : (j + 1) * 128], ident)
                with tc.high_priority():
                    nc.vector.tensor_copy(v_sb[stile], pst)

    # ---- stage C: attention for q-chunk qc.  Diagonal blocks restricted to
    # the valid causal q-range, interleaved with full off-diag blocks; the
    # PV/denominator matmuls run 2 blocks behind the scores. ----
    def attn_chunk(qc):
        diags = list(range(4 * qc, 4 * (qc + 1)))
        offs = list(range(0, 4 * qc))
        order = []
        for i in range(max(len(diags), len(offs))):
            if i < len(offs):
                order.append(offs[i])
            if i < len(diags):
                order.append(diags[i])
        ot_tiles = {}
        for h in range(GQ):
            psum_o = psO.tile([128, 512], F32, tag="pO", name=f"pso{qc}_{h}")
            psum_d = psD.tile([1, 512], F32, tag="pD", name=f"psd{qc}_{h}")
            first_kt = order[0]
            pend = []

            def flush_one(is_last):
                pkt, poff, pw, pexp = pend.pop(0)
                st = pkt == first_kt
                nc.tensor.matmul(psum_o[:, poff : poff + pw], v_sb[pkt],
                                 pexp[:, 0:pw], start=st, stop=is_last,
                                 skip_group_check=True)
                nc.tensor.matmul(psum_d[:, poff : poff + pw], ones_col,
                                 pexp[:, 0:pw], start=st, stop=is_last,
                                 skip_group_check=True)

            for kt in order:
                diag = kt >= 4 * qc
                off = (kt - 4 * qc) * 128 if diag else 0
                w = 512 - off
                ps_s = psS.tile([128, 512], F32, tag="pS",
                                name=f"pss{qc}_{h}_{kt}")
                nc.tensor.matmul(
                    ps_s[:, 0:w],
                    kfin[:, kt * 128 : (kt + 1) * 128],
                    qfin[h][:, qc * 512 + off : (qc + 1) * 512],
                    start=True, stop=True)
                exp_s = expp.tile([128, 512], BF16, tag="exp")
                with tc.high_priority():
                    nc.scalar.activation(exp_s[:, 0:w], ps_s[:, 0:w], AF.Exp)
                if diag:  # keep q >= k: col j >= partition p
                    nc.vector.tensor_mul(exp_s[:, 0:w], exp_s[:, 0:w],
                                         mask_sb[:, 0:w])
                pend.append((kt, off, w, exp_s))
                if len(pend) > 4:
                    flush_one(False)
            while pend:
                flush_one(len(pend) == 1)
            # normalize: O.T = O'.T * (1/denom) broadcast across partitions
            rf_row = rowp.tile([1, 512], F32, tag="rfr", bufs=2)
            nc.vector.reciprocal_approx_fast(rf_row, psum_d)
            rb = bpool.tile([128, 512], F32, tag="rb", bufs=2)
            nc.gpsimd.partition_broadcast(rb, rf_row)
            ot = otp.tile([128, 512], BF16, tag=f"ot{h}", bufs=1)
            nc.vector.tensor_mul(ot, psum_o, rb)
            ot_tiles[h] = ot
        return ot_tiles

    # ---- stage D: output projection for q-chunk qc ----
    def oproj_chunk(qc, ot_tiles):
        for etg in range(8):
            r = resp.tile([128, 2, 512], F32, tag="res")
            for e2 in range(2):
                pool_r = psS if etg % 2 == 0 else psO
                ps_res = pool_r.tile([128, 512], F32,
                                     tag="pS" if etg % 2 == 0 else "pO",
                                     name=f"psres{qc}_{etg}_{e2}")
                for h in range(GQ):
                    nc.tensor.matmul(
                        ps_res, wo_sb[:, etg, h, e2 * 128 : (e2 + 1) * 128],
                        ot_tiles[h],
                        start=(h == 0), stop=(h == GQ - 1))
                if e2 == 0:
                    nc.vector.tensor_copy(r[:, e2, :], ps_res)
                else:
                    nc.scalar.activation(r[:, e2, :], ps_res, AF.Identity)
                if qc == SC - 1:
                    nc.sync.dma_start(
                        bass.AP(tensor=resT.tensor,
                                offset=(etg * 2 + e2) * 128 * S + qc * 512,
                                ap=[[S, 128], [1, 512]]),
                        r[:, e2, :])
            if qc != SC - 1:
                nc.sync.dma_start(
                    bass.AP(tensor=resT.tensor,
                            offset=etg * 2 * 128 * S + qc * 512,
                            ap=[[S, 128], [128 * S, 2], [1, 512]]),
                    r)

    # ---- main schedule: proj(sc) | vts | [x prefetch] | attn(sc) | oproj ----
    xq = emit_x_chunk(0)
    for sc in range(SC):
        proj_chunk(sc, xq)
        flush_vts()
        # prefetch the exp ACT table while the PE runs the pass-2 tail +
        # v transposes, so attention's first real exp doesn't eat the
        # 1.3us table reload
        with tc.high_priority():
            nc.scalar.activation(scratch, sce[0:1, 0:1], AF.Exp)
        if sc == 0:
            emit_wo_dmas()
        if sc + 1 < SC:
            xq = emit_x_chunk(sc + 1)
        ot_tiles = attn_chunk(sc)
        oproj_chunk(sc, ot_tiles)

    ctx.close()


def _build():
    global _COMPILED_NC
    if _COMPILED_NC is not None:
        return _COMPILED_NC
    nc = bacc.Bacc("TRN2", target_bir_lowering=False, debug=False,
                   num_devices=NCORES)
    with tile.TileContext(nc) as tc:
        _build_body(tc)
    nc.compile()
    _COMPILED_NC = nc
    return nc


def _rope_tables():
    import ml_dtypes
    inv_freq = 1.0 / (ROPE_BASE ** (np.arange(0, HD, 2, dtype=np.float64) / HD))
    t = np.arange(S, dtype=np.float64)
    freqs = np.outer(t, inv_freq)          # [S, 64]
    cos = np.cos(freqs).T
    sin = np.sin(freqs).T
    cs = np.concatenate([cos, cos], axis=0).astype(ml_dtypes.bfloat16)
    sn = np.concatenate([sin, -sin], axis=0).astype(ml_dtypes.bfloat16)
    return np.ascontiguousarray(cs), np.ascontiguousarray(sn)


def kernel(x, Wq, bq, Wk, bk, Wv, bv, Wo, bo, q_gain):
    import ml_dtypes
    bf16 = ml_dtypes.bfloat16
    x = np.asarray(x, np.float32)
    Wq = np.asarray(Wq, np.float32)
    bq = np.asarray(bq, np.float32)
    Wk = np.asarray(Wk, np.float32)
    bk = np.asarray(bk, np.float32)
    Wv = np.asarray(Wv, np.float32)
    bv = np.asarray(bv, np.float32)
    Wo = np.asarray(Wo, np.float32)
    bo = np.asarray(bo, np.float32)
    q_gain = np.asarray(q_gain, np.float32)

    cs, sn = _rope_tables()
    ones_arr = np.ones((128, 1), bf16)
    mask_arr = np.ascontiguousarray(
        (np.arange(512)[None, :] >= np.arange(128)[:, None]).astype(bf16))
    ident_arr = np.eye(128, dtype=bf16)
    xb = [np.ascontiguousarray(x[b].T.astype(bf16)) for b in range(B)]

    def sced_arr(g):
        # rsqrt scale/bias: g*rsqrt(ss/HD+eps) == rsqrt(ss*A+B)
        gg = np.concatenate([g.astype(np.float64), [HD ** -0.5]])
        a = 1.0 / (HD * gg * gg)
        b = EPS / (gg * gg)
        row = np.concatenate([a, b]).astype(np.float32)   # [2*(GQ+1)]
        return np.ascontiguousarray(np.broadcast_to(row, (128, row.size)))

    in_maps = []
    for c in range(NCORES):
        b, g = divmod(c, KVH)
        sl = slice(g * SL, (g + 1) * SL)
        hs = slice(g * HD, (g + 1) * HD)
        in_maps.append({
            "xT": xb[b],
            "wqT": np.ascontiguousarray(Wq[sl, :].T.astype(bf16)),
            "wkT": np.ascontiguousarray(Wk[hs, :].T.astype(bf16)),
            "wvT": np.ascontiguousarray(Wv[hs, :].T.astype(bf16)),
            "woA": np.ascontiguousarray(Wo[:, sl].T.astype(bf16)),
            "csd": cs, "snd": sn,
            "bqkd": np.ascontiguousarray(np.concatenate(
                [bq[sl].reshape(GQ, HD).T, bk[hs].reshape(1, HD).T], axis=1)),
            "sced": sced_arr(q_gain[g * GQ : (g + 1) * GQ]),
            "onesd": ones_arr, "identd": ident_arr, "maskd": mask_arr,
        })

    global _LAST_IN_MAPS
    _LAST_IN_MAPS = in_maps
    nc = _build()
    res = bass_utils.run_bass_kernel_spmd(nc, in_maps, core_ids=list(range(NCORES)))

    # v-bias and o-bias folded on host: attention rows sum to 1, so +bv
    # passes through to O exactly; res += bv_rep @ Wo.T + bo.
    bv_rep = np.repeat(bv.reshape(KVH, HD), H // KVH, axis=0).reshape(-1)
    host_const = (Wo @ bv_rep + bo).astype(np.float32)

    out = np.zeros((B, S, D), np.float32)
    for c in range(NCORES):
        b = c // KVH
        out[b] += res.results[c]["resT"].T
    out += host_const[None, None, :]
    return out
